# revision 47
# baseline (speedup 1.0000x reference)
"""TRN2 Bass kernel: multi-head attention (b=16, s=1024, d=512, h=8).

Sharding: data-parallel over batch — 16 batches across 8 NeuronCores, 2 per
core, no collectives. Returns (output, attention_weights) like the reference.

Per-core dataflow (all matmuls in fp32r: 12-bit mantissa, 1 cyc/row on PE):
  stage01: load q/k/v naturally, PE-transpose to x^T, project:
           Q^T,K^T [512, s] (transposed), V [s, 512] (natural).
  phase C (per head-pair): S = Q_h K_h^T row-packed on the PE (K=64 pairs in
           row groups 0-63/64-127), exp on ScalarE with row-sum accum_out,
           normalize+reciprocal on GpSimd (normalize_recip), DMA attention
           weights to HBM.
  phase A: recompute S^T = K_h Q_h^T (row-packed, head pair shares one PSUM
           tile), exp -> fp32r, PV matmuls accumulate ctx^T per head; ctx^T
           normalized during PSUM->SBUF copy (tensor_mul with a reciprocal
           row broadcast via a DRAM bounce).
  phase D: output projection from stacked ctx^T chunks, scaled rows done.
"""
import numpy as np

B_FULL, S, D, H, DH, P = 16, 1024, 512, 8, 64, 128
NCORES = 8
B = B_FULL // NCORES   # batches per core
NT = S // P            # seq tiles
NC = D // P            # d_model chunks

USE_GPSIMD_NORMALIZE = False

_STATE = {}


def _round_fp32r(x):
    """Round fp32 to fp32r (keep 12 mantissa bits, round-to-nearest-even).

    Bit-exact with the DVE/walrus cast (verified vs hardware tensor_scalar).
    """
    u = np.ascontiguousarray(x, dtype=np.float32).view(np.uint32)
    lo = u & np.uint32(0x00000FFF)
    base = u & np.uint32(0xFFFFF000)
    rnd = np.where(
        (lo > 0x800) | ((lo == 0x800) & (((u >> np.uint32(12)) & np.uint32(1)) == 1)),
        base + np.uint32(0x1000),
        base,
    ).astype(np.uint32)
    return rnd.view(np.float32).reshape(np.shape(x))


def _build_module():
    from contextlib import ExitStack

    import concourse.bass as bass
    import concourse.tile as tile
    from concourse import bacc, mybir

    F32 = mybir.dt.float32
    F32R = mybir.dt.float32r
    EXP = mybir.ActivationFunctionType.Exp

    nc = bacc.Bacc("TRN2", target_bir_lowering=False, debug=False,
                   num_devices=NCORES)

    q_d = nc.dram_tensor("q", [B, D, S], F32R, kind="ExternalInput").ap()
    k_d = nc.dram_tensor("k", [B, D, S], F32R, kind="ExternalInput").ap()
    v_d = nc.dram_tensor("v", [B, D, S], F32R, kind="ExternalInput").ap()
    wq_d = nc.dram_tensor("wq", [D, D], F32R, kind="ExternalInput").ap()
    wk_d = nc.dram_tensor("wk", [D, D], F32R, kind="ExternalInput").ap()
    wv_d = nc.dram_tensor("wv", [D, D], F32R, kind="ExternalInput").ap()
    wo_d = nc.dram_tensor("wo", [D, D], F32R, kind="ExternalInput").ap()
    out_d = nc.dram_tensor("out", [B, S, D], F32, kind="ExternalOutput").ap()
    attn_d = nc.dram_tensor("attn", [B, H, S, S], F32,
                            kind="ExternalOutput").ap()

    with tile.TileContext(nc) as tc, ExitStack() as ctx:
        sb = ctx.enter_context(tc.tile_pool(name="sb", bufs=2))
        psA = ctx.enter_context(tc.tile_pool(name="psA", bufs=2, space="PSUM"))
        psB = ctx.enter_context(tc.tile_pool(name="psB", bufs=2, space="PSUM"))
        dramp = ctx.enter_context(tc.tile_pool(name="dramp", bufs=8,
                                               space="DRAM"))

        w_sb = {}
        _wd = {"wq": wq_d, "wk": wk_d, "wv": wv_d, "wo": wo_d}

        def load_w(name):
            if name in w_sb:
                return
            d = _wd[name]
            tiles = []
            for c in range(NC):
                t = sb.tile([P, D], F32R, tag="w", bufs=16,
                            name=f"w{name}{c}")
                nc.sync.dma_start(t, d[c * P:(c + 1) * P, :])
                tiles.append(t)
            w_sb[name] = tiles

        QT = [[None] * NC for _ in range(B)]
        KT = [[None] * NC for _ in range(B)]
        V = [[None] * NT for _ in range(B)]
        ctxTr = [[None] * (H // 2) for _ in range(B)]

        def stage01(b):
            """Load pre-transposed x^T and run projections."""
            for sel, x_d in (("q", q_d), ("k", k_d), ("v", v_d)):
                xT = []
                for c in range(NC):
                    t = sb.tile([P, S], F32R, tag="xT", bufs=7,
                                name=f"xT{sel}{c}")
                    eng = nc.sync if (b == 0 and sel == "q") else nc.gpsimd
                    eng.dma_start(t, x_d[b, c * P:(c + 1) * P, :])
                    xT.append(t)
                if sel in ("q", "k"):
                    load_w("wq")
                    load_w("wk")
                    Wt = w_sb["wq" if sel == "q" else "wk"]
                    dest = QT[b] if sel == "q" else KT[b]
                    for oc in range(NC):
                        dst = sb.tile([P, S], F32R, tag="QT", bufs=9)
                        if b == 0:
                            ps = psA.tile([P, S], F32, tag="dual", bufs=2)
                            for ic in range(NC):
                                for hh in range(2):
                                    nc.tensor.matmul(
                                        ps[:, hh * D:(hh + 1) * D],
                                        Wt[ic][:, oc * P:(oc + 1) * P],
                                        xT[ic][:, hh * D:(hh + 1) * D],
                                        start=(ic == 0), stop=(ic == NC - 1))
                            nc.vector.tensor_copy(dst, ps)
                        else:
                            for hh in range(2):
                                ps = psB.tile([P, D], F32, tag="ps1", bufs=1)
                                for ic in range(NC):
                                    nc.tensor.matmul(
                                        ps,
                                        Wt[ic][:, oc * P:(oc + 1) * P],
                                        xT[ic][:, hh * D:(hh + 1) * D],
                                        start=(ic == 0), stop=(ic == NC - 1))
                                nc.vector.tensor_copy(
                                    dst[:, hh * D:(hh + 1) * D], ps)
                        dest[oc] = dst
                else:
                    load_w("wv")
                    load_w("wo")
                    for t in range(NT):
                        ps = psB.tile([P, D], F32, tag="ps1", bufs=1)
                        for ic in range(NC):
                            nc.tensor.matmul(
                                ps, xT[ic][:, t * P:(t + 1) * P],
                                w_sb["wv"][ic],
                                start=(ic == 0), stop=(ic == NC - 1))
                        dst = sb.tile([P, D], F32R, tag="V", bufs=8)
                        nc.vector.tensor_copy(dst, ps)
                        V[b][t] = dst

        def phaseC_tile(b, hp, t, rs, scr, Rt):
            qt, kt = QT[b][hp], KT[b][hp]
            lo, hi = slice(0, DH), slice(DH, P)
            sAB = {}
            for head in (2 * hp, 2 * hp + 1):
                sAB[head] = psA.tile([P, S], F32, tag="dual", bufs=2,
                                     name=f"s{b}_{head}_{t}")
            qA = qt[lo, t * P:(t + 1) * P]
            qB = qt[hi, t * P:(t + 1) * P]
            # row-packed pairs: head A rows 0-63, head B rows 64-127
            nc.tensor.matmul(sAB[2 * hp][:, 0:D], qA, kt[lo, 0:D],
                             start=True, stop=True)
            nc.tensor.matmul(sAB[2 * hp + 1][:, 0:D], qB, kt[hi, 0:D],
                             start=True, stop=True)
            nc.tensor.matmul(sAB[2 * hp][:, D:S], qA, kt[lo, D:S],
                             start=True, stop=True)
            nc.tensor.matmul(sAB[2 * hp + 1][:, D:S], qB, kt[hi, D:S],
                             start=True, stop=True)
            for head in (2 * hp, 2 * hp + 1):
                p = sb.tile([P, S], F32, tag="P", bufs=8)
                rcol = rs[head][:, t:t + 1]
                if head % 2 == 0:
                    nc.scalar.activation(p, sAB[head], EXP, accum_out=rcol)
                else:
                    nc.scalar.activation(p, sAB[head], EXP)
                    nc.vector.tensor_reduce(
                        rcol, p, axis=mybir.AxisListType.X,
                        op=mybir.AluOpType.add)
                nc.vector.reciprocal(rcol, rcol)
                nc.vector.tensor_scalar_mul(p, p, rcol)
                nc.sync.dma_start(
                    attn_d[b, head, t * P:(t + 1) * P, :], p)
                if t in (3, NT - 1):
                    # batched transpose-write of 4 recip columns, then
                    # broadcast-read that half of the recip row
                    half = 0 if t == 3 else 1
                    nc.gpsimd.dma_start(
                        scr[head][half * 4:half * 4 + 4, :]
                        .rearrange("a b -> b a"),
                        rs[head][:, half * 4:half * 4 + 4])
                    flat = scr[head].rearrange("t p -> (t p)")
                    src_ap = bass.AP(
                        tensor=flat.tensor, offset=flat.offset + half * D,
                        ap=[[0, DH], [1, D]])
                    nc.gpsimd.dma_start(
                        Rt[head][:, half * D:(half + 1) * D], src_ap)

        def phaseA_chunk(b, hp, ss, c, ctxA, ctxB):
            qt, kt = QT[b][hp], KT[b][hp]
            hA, hB = 2 * hp, 2 * hp + 1
            lo, hi = slice(0, DH), slice(DH, P)
            st = psA.tile([P, S], F32, tag="dual", bufs=2,
                          name=f"st{b}_{hp}_{ss}_{c}")
            nc.tensor.matmul(
                st[:, 0:D], kt[lo, c * P:(c + 1) * P],
                qt[lo, ss * D:(ss + 1) * D], start=True, stop=True)
            nc.tensor.matmul(
                st[:, D:S], kt[hi, c * P:(c + 1) * P],
                qt[hi, ss * D:(ss + 1) * D], start=True, stop=True)
            ex = sb.tile([P, S], F32R, tag="E", bufs=3,
                         name=f"ex{b}_{hp}_{ss}_{c}")
            nc.scalar.activation(ex, st, EXP)
            nc.tensor.matmul(
                ctxA, V[b][c][:, hA * DH:(hA + 1) * DH], ex[:, 0:D],
                start=(c == 0), stop=(c == NT - 1))
            nc.tensor.matmul(
                ctxB, V[b][c][:, hB * DH:(hB + 1) * DH], ex[:, D:S],
                start=(c == 0), stop=(c == NT - 1))

        def ctx_norm(b, hp, ss, ctxA, ctxB, Rt):
            hA, hB = 2 * hp, 2 * hp + 1
            lo, hi = slice(0, DH), slice(DH, P)
            cT = ctxTr[b][hp]
            nc.vector.tensor_mul(
                cT[lo, ss * D:(ss + 1) * D], ctxA,
                Rt[hA][:, ss * D:(ss + 1) * D])
            nc.vector.tensor_mul(
                cT[hi, ss * D:(ss + 1) * D], ctxB,
                Rt[hB][:, ss * D:(ss + 1) * D])

        def build_R(b, hp, scr):
            """Broadcast recip sums to [64, S] rows via DRAM replicate-read."""
            Rt = {}
            for head in (2 * hp, 2 * hp + 1):
                Rh = sb.tile([DH, S], F32, tag="R", bufs=2,
                             name=f"R{b}_{head}")
                flat = scr[head].rearrange("t p -> (t p)")
                src_ap = bass.AP(
                    tensor=flat.tensor, offset=flat.offset,
                    ap=[[0, DH]] + [list(dim) for dim in flat.ap])
                nc.gpsimd.dma_start(Rh, src_ap)
                Rt[head] = Rh
            return Rt

        def phaseD(b):
            for t in range(NT):
                ps = psB.tile([P, D], F32, tag="ps1", bufs=1)
                for c in range(NC):
                    nc.tensor.matmul(
                        ps, ctxTr[b][c][:, t * P:(t + 1) * P], w_sb["wo"][c],
                        start=(c == 0), stop=(c == NC - 1))
                osb = sb.tile([P, D], F32, tag="osb", bufs=2)
                if b == 0:
                    nc.vector.tensor_copy(osb, ps)
                else:
                    nc.scalar.copy(osb, ps)
                nc.sync.dma_start(out_d[b, t * P:(t + 1) * P, :], osb)

        def attn_unit(b, hp):
            rs = {}
            scr = {}
            Rt = {}
            for head in (2 * hp, 2 * hp + 1):
                rs[head] = sb.tile([P, NT], F32, tag="rs", bufs=4,
                                   name=f"rs{b}_{head}")
                scr[head] = dramp.tile([NT, P], F32, name=f"scr{b}_{head}")
                Rt[head] = sb.tile([DH, S], F32, tag="R", bufs=2,
                                   name=f"R{b}_{head}")
            cT = sb.tile([P, S], F32R, tag="ctxT", bufs=6,
                         name=f"cT{b}_{hp}")
            ctxTr[b][hp] = cT
            ctx0 = [psB.tile([DH, D], F32, tag="ctx", bufs=3,
                             name=f"ctx0{b}_{hp}_{i}") for i in range(2)]
            ctx1 = [psB.tile([DH, D], F32, tag="ctx", bufs=3,
                             name=f"ctx1{b}_{hp}_{i}") for i in range(2)]
            for t in range(NT):
                phaseC_tile(b, hp, t, rs, scr, Rt)
                if t >= 6:
                    phaseA_chunk(b, hp, 0, t - 6, ctx0[0], ctx0[1])
            for c in range(2, NT):
                phaseA_chunk(b, hp, 0, c, ctx0[0], ctx0[1])
            for c in range(NT):
                phaseA_chunk(b, hp, 1, c, ctx1[0], ctx1[1])
            ctx_norm(b, hp, 0, ctx0[0], ctx0[1], Rt)
            ctx_norm(b, hp, 1, ctx1[0], ctx1[1], Rt)

        stage01(0)
        attn_unit(0, 0)
        attn_unit(0, 1)
        stage01(1)
        attn_unit(0, 2)
        attn_unit(0, 3)
        phaseD(0)
        for hp in range(H // 2):
            attn_unit(1, hp)
        phaseD(1)

    nc.compile()
    return nc


def _make_runner(nc):
    """Cached PJRT runner mirroring bass2jax.run_bass_via_pjrt (multi-core)."""
    import jax
    from concourse import bass2jax, mybir
    from jax.experimental.shard_map import shard_map
    from jax.sharding import Mesh, PartitionSpec

    bass2jax.install_neuronx_cc_hook()
    partition_name = (nc.partition_id_tensor.name
                      if nc.partition_id_tensor else None)
    in_names, out_names, out_avals, zero_shapes = [], [], [], []
    for alloc in nc.m.functions[0].allocations:
        if not isinstance(alloc, mybir.MemoryLocationSet):
            continue
        name = alloc.memorylocations[0].name
        if alloc.kind == "ExternalInput":
            if name != partition_name:
                in_names.append(name)
        elif alloc.kind == "ExternalOutput":
            out_names.append(name)
            shape = tuple(alloc.tensor_shape)
            dtype = mybir.dt.np(alloc.dtype)
            out_avals.append(jax.core.ShapedArray(shape, dtype))
            zero_shapes.append((shape, dtype))
    n_params = len(in_names)
    all_in = list(in_names) + list(out_names) + (
        [partition_name] if partition_name else [])
    donate = tuple(range(n_params, n_params + len(out_names)))

    def _body(*args):
        operands = list(args)
        if partition_name:
            operands.append(bass2jax.partition_id_tensor())
        outs = bass2jax._bass_exec_p.bind(
            *operands, out_avals=tuple(out_avals), in_names=tuple(all_in),
            out_names=tuple(out_names), lowering_input_output_aliases=(),
            sim_require_finite=True, sim_require_nnan=True, nc=nc)
        return tuple(outs)

    devices = jax.devices()[:NCORES]
    mesh = Mesh(np.asarray(devices), ("core",))
    specs_in = (PartitionSpec("core"),) * (n_params + len(out_names))
    specs_out = (PartitionSpec("core"),) * len(out_names)
    sharded = jax.jit(
        shard_map(_body, mesh=mesh, in_specs=specs_in, out_specs=specs_out,
                  check_rep=False),
        donate_argnums=donate, keep_unused=True)

    def run(in_maps):
        concat_in = [
            np.concatenate([np.asarray(m[nm]) for m in in_maps], axis=0)
            for nm in in_names]
        concat_zeros = [
            np.zeros((NCORES * sh[0], *sh[1:]), dt) for sh, dt in zero_shapes]
        outs = sharded(*concat_in, *concat_zeros)
        outs = [np.asarray(o) for o in outs]
        return dict(zip(out_names, outs))

    return run


def get_runner():
    if "run" not in _STATE:
        nc = _build_module()
        _STATE["nc"] = nc
        _STATE["run"] = _make_runner(nc)
    return _STATE["run"]


def _numpy_fallback(v, k, q, mask, Wq, Wk, Wv, Wo):
    """Reference implementation in numpy (used only if mask isn't all-ones)."""
    b = q.shape[0]
    scale = 1.0 / np.sqrt(np.float32(DH))

    def split_heads(x):
        return x.reshape(b, S, H, DH).transpose(0, 2, 1, 3)

    qh = split_heads(q @ Wq)
    kh = split_heads(k @ Wk)
    vh = split_heads(v @ Wv)
    logits = np.einsum("bhqd,bhkd->bhqk", qh, kh) * scale
    logits = logits + (1.0 - mask[:, None, None, :]) * (-1e9)
    logits = logits - logits.max(-1, keepdims=True)
    e = np.exp(logits)
    attn = e / e.sum(-1, keepdims=True)
    ctx = np.einsum("bhqk,bhkd->bhqd", attn, vh)
    ctx = ctx.transpose(0, 2, 1, 3).reshape(b, S, D)
    return (ctx @ Wo).astype(np.float32), attn.astype(np.float32)


def kernel(**inputs):
    q = np.asarray(inputs["q"], np.float32)
    k = np.asarray(inputs["k"], np.float32)
    v = np.asarray(inputs["v"], np.float32)
    mask = np.asarray(inputs["mask"], np.float32)
    Wq = np.asarray(inputs["Wq"], np.float32)
    Wk = np.asarray(inputs["Wk"], np.float32)
    Wv = np.asarray(inputs["Wv"], np.float32)
    Wo = np.asarray(inputs["Wo"], np.float32)

    if not np.all(mask == 1.0):
        return _numpy_fallback(v, k, q, mask, Wq, Wk, Wv, Wo)

    run = get_runner()

    qr = np.ascontiguousarray(
        _round_fp32r(q * np.float32(1.0 / np.sqrt(np.float32(DH))))
        .transpose(0, 2, 1))
    kr = np.ascontiguousarray(_round_fp32r(k).transpose(0, 2, 1))
    vr = np.ascontiguousarray(_round_fp32r(v).transpose(0, 2, 1))
    wqr = _round_fp32r(Wq)
    wkr = _round_fp32r(Wk)
    wvr = _round_fp32r(Wv)
    wor = _round_fp32r(Wo)
    in_maps = []
    for c in range(NCORES):
        sl = slice(c * B, (c + 1) * B)
        in_maps.append({
            "q": qr[sl], "k": kr[sl], "v": vr[sl],
            "wq": wqr, "wk": wkr, "wv": wvr, "wo": wor,
        })
    outs = run(in_maps)
    output = outs["out"].reshape(B_FULL, S, D)
    attn = outs["attn"].reshape(B_FULL, H, S, S)
    return output, attn


# revision 56
# speedup vs baseline: 1.0136x; 1.0136x over previous
"""TRN2 Bass kernel: multi-head attention (b=16, s=1024, d=512, h=8).

Sharding: data-parallel over batch — 16 batches across 8 NeuronCores, 2 per
core, no collectives. Returns (output, attention_weights) like the reference.

Per-core dataflow (all matmuls in fp32r: 12-bit mantissa, 1 cyc/row on PE):
  stage01: load q/k/v naturally, PE-transpose to x^T, project:
           Q^T,K^T [512, s] (transposed), V [s, 512] (natural).
  phase C (per head-pair): S = Q_h K_h^T row-packed on the PE (K=64 pairs in
           row groups 0-63/64-127), exp on ScalarE with row-sum accum_out,
           normalize+reciprocal on GpSimd (normalize_recip), DMA attention
           weights to HBM.
  phase A: recompute S^T = K_h Q_h^T (row-packed, head pair shares one PSUM
           tile), exp -> fp32r, PV matmuls accumulate ctx^T per head; ctx^T
           normalized during PSUM->SBUF copy (tensor_mul with a reciprocal
           row broadcast via a DRAM bounce).
  phase D: output projection from stacked ctx^T chunks, scaled rows done.
"""
import numpy as np

B_FULL, S, D, H, DH, P = 16, 1024, 512, 8, 64, 128
NCORES = 8
B = B_FULL // NCORES   # batches per core
NT = S // P            # seq tiles
NC = D // P            # d_model chunks

USE_GPSIMD_NORMALIZE = False

_STATE = {}


def _round_fp32r(x):
    """Round fp32 to fp32r (keep 12 mantissa bits, round-to-nearest-even).

    Bit-exact with the DVE/walrus cast (verified vs hardware tensor_scalar).
    """
    u = np.ascontiguousarray(x, dtype=np.float32).view(np.uint32)
    lo = u & np.uint32(0x00000FFF)
    base = u & np.uint32(0xFFFFF000)
    rnd = np.where(
        (lo > 0x800) | ((lo == 0x800) & (((u >> np.uint32(12)) & np.uint32(1)) == 1)),
        base + np.uint32(0x1000),
        base,
    ).astype(np.uint32)
    return rnd.view(np.float32).reshape(np.shape(x))


def _build_module():
    from contextlib import ExitStack

    import concourse.bass as bass
    import concourse.tile as tile
    from concourse import bacc, mybir

    F32 = mybir.dt.float32
    F32R = mybir.dt.float32r
    EXP = mybir.ActivationFunctionType.Exp

    nc = bacc.Bacc("TRN2", target_bir_lowering=False, debug=False,
                   num_devices=NCORES)

    q_d = nc.dram_tensor("q", [B, D, S], F32R, kind="ExternalInput").ap()
    k_d = nc.dram_tensor("k", [B, D, S], F32R, kind="ExternalInput").ap()
    v_d = nc.dram_tensor("v", [B, D, S], F32R, kind="ExternalInput").ap()
    wq_d = nc.dram_tensor("wq", [D, D], F32R, kind="ExternalInput").ap()
    wk_d = nc.dram_tensor("wk", [D, D], F32R, kind="ExternalInput").ap()
    wv_d = nc.dram_tensor("wv", [D, D], F32R, kind="ExternalInput").ap()
    wo_d = nc.dram_tensor("wo", [D, D], F32R, kind="ExternalInput").ap()
    out_d = nc.dram_tensor("out", [B, S, D], F32, kind="ExternalOutput").ap()
    attn_d = nc.dram_tensor("attn", [B, H, S, S], F32,
                            kind="ExternalOutput").ap()

    with tile.TileContext(nc) as tc, ExitStack() as ctx:
        sb = ctx.enter_context(tc.tile_pool(name="sb", bufs=2))
        psA = ctx.enter_context(tc.tile_pool(name="psA", bufs=2, space="PSUM"))
        psB = ctx.enter_context(tc.tile_pool(name="psB", bufs=2, space="PSUM"))
        dramp = ctx.enter_context(tc.tile_pool(name="dramp", bufs=8,
                                               space="DRAM"))

        w_sb = {}
        _wd = {"wq": wq_d, "wk": wk_d, "wv": wv_d, "wo": wo_d}

        def load_w(name):
            if name in w_sb:
                return
            d = _wd[name]
            tiles = []
            for c in range(NC):
                t = sb.tile([P, D], F32R, tag="w", bufs=16,
                            name=f"w{name}{c}")
                nc.sync.dma_start(t, d[c * P:(c + 1) * P, :])
                tiles.append(t)
            w_sb[name] = tiles

        QT = [[None] * NC for _ in range(B)]
        KT = [[None] * NC for _ in range(B)]
        V = [[None] * NT for _ in range(B)]
        ctxTr = [[None] * (H // 2) for _ in range(B)]

        def stage01(b):
            """Load pre-transposed x^T and run projections."""
            for sel, x_d in (("q", q_d), ("k", k_d), ("v", v_d)):
                xT = []
                for c in range(NC):
                    t = sb.tile([P, S], F32R, tag="xT", bufs=7,
                                name=f"xT{sel}{c}")
                    eng = nc.sync if (b == 0 and sel == "q") else nc.gpsimd
                    eng.dma_start(t, x_d[b, c * P:(c + 1) * P, :])
                    xT.append(t)
                if sel in ("q", "k"):
                    load_w("wq")
                    load_w("wk")
                    Wt = w_sb["wq" if sel == "q" else "wk"]
                    dest = QT[b] if sel == "q" else KT[b]
                    for oc in range(NC):
                        dst = sb.tile([P, S], F32R, tag="QT", bufs=9)
                        if b == 0:
                            ps = psA.tile([P, S], F32, tag="dual", bufs=2)
                            for ic in range(NC):
                                for hh in range(2):
                                    nc.tensor.matmul(
                                        ps[:, hh * D:(hh + 1) * D],
                                        Wt[ic][:, oc * P:(oc + 1) * P],
                                        xT[ic][:, hh * D:(hh + 1) * D],
                                        start=(ic == 0), stop=(ic == NC - 1))
                            nc.vector.tensor_copy(dst, ps)
                        else:
                            for hh in range(2):
                                ps = psB.tile([P, D], F32, tag="ps1", bufs=1)
                                for ic in range(NC):
                                    nc.tensor.matmul(
                                        ps,
                                        Wt[ic][:, oc * P:(oc + 1) * P],
                                        xT[ic][:, hh * D:(hh + 1) * D],
                                        start=(ic == 0), stop=(ic == NC - 1))
                                nc.vector.tensor_copy(
                                    dst[:, hh * D:(hh + 1) * D], ps)
                        dest[oc] = dst
                else:
                    load_w("wv")
                    load_w("wo")
                    for t in range(NT):
                        ps = psB.tile([P, D], F32, tag="ps1", bufs=1)
                        for ic in range(NC):
                            nc.tensor.matmul(
                                ps, xT[ic][:, t * P:(t + 1) * P],
                                w_sb["wv"][ic],
                                start=(ic == 0), stop=(ic == NC - 1))
                        dst = sb.tile([P, D], F32R, tag="V", bufs=8)
                        nc.vector.tensor_copy(dst, ps)
                        V[b][t] = dst

        def phaseC_tile(b, hp, t, rs, scr, Rt):
            qt, kt = QT[b][hp], KT[b][hp]
            lo, hi = slice(0, DH), slice(DH, P)
            sAB = {}
            for head in (2 * hp, 2 * hp + 1):
                sAB[head] = psA.tile([P, S], F32, tag="dual", bufs=2,
                                     name=f"s{b}_{head}_{t}")
            qA = qt[lo, t * P:(t + 1) * P]
            qB = qt[hi, t * P:(t + 1) * P]
            # row-packed pairs: head A rows 0-63, head B rows 64-127
            nc.tensor.matmul(sAB[2 * hp][:, 0:D], qA, kt[lo, 0:D],
                             start=True, stop=True)
            nc.tensor.matmul(sAB[2 * hp + 1][:, 0:D], qB, kt[hi, 0:D],
                             start=True, stop=True)
            nc.tensor.matmul(sAB[2 * hp][:, D:S], qA, kt[lo, D:S],
                             start=True, stop=True)
            nc.tensor.matmul(sAB[2 * hp + 1][:, D:S], qB, kt[hi, D:S],
                             start=True, stop=True)
            for head in (2 * hp, 2 * hp + 1):
                p = sb.tile([P, S], F32, tag="P", bufs=8)
                rcol = rs[head][:, t:t + 1]
                if head % 2 == 0:
                    nc.scalar.activation(p, sAB[head], EXP, accum_out=rcol)
                else:
                    nc.scalar.activation(p, sAB[head], EXP)
                    nc.vector.tensor_reduce(
                        rcol, p, axis=mybir.AxisListType.X,
                        op=mybir.AluOpType.add)
                nc.vector.reciprocal(rcol, rcol)
                nc.vector.tensor_scalar_mul(p, p, rcol)
                nc.sync.dma_start(
                    attn_d[b, head, t * P:(t + 1) * P, :], p)
                if t in (3, NT - 1):
                    # batched transpose-write of 4 recip columns, then
                    # broadcast-read that half of the recip row
                    half = 0 if t == 3 else 1
                    nc.gpsimd.dma_start(
                        scr[head][half * 4:half * 4 + 4, :]
                        .rearrange("a b -> b a"),
                        rs[head][:, half * 4:half * 4 + 4])
                    flat = scr[head].rearrange("t p -> (t p)")
                    src_ap = bass.AP(
                        tensor=flat.tensor, offset=flat.offset + half * D,
                        ap=[[0, DH], [1, D]])
                    nc.gpsimd.dma_start(
                        Rt[head][:, half * D:(half + 1) * D], src_ap)

        def phaseA_chunk(b, hp, ss, c, ctxA, ctxB):
            qt, kt = QT[b][hp], KT[b][hp]
            hA, hB = 2 * hp, 2 * hp + 1
            lo, hi = slice(0, DH), slice(DH, P)
            st = psA.tile([P, S], F32, tag="dual", bufs=2,
                          name=f"st{b}_{hp}_{ss}_{c}")
            nc.tensor.matmul(
                st[:, 0:D], kt[lo, c * P:(c + 1) * P],
                qt[lo, ss * D:(ss + 1) * D], start=True, stop=True)
            nc.tensor.matmul(
                st[:, D:S], kt[hi, c * P:(c + 1) * P],
                qt[hi, ss * D:(ss + 1) * D], start=True, stop=True)
            ex = sb.tile([P, S], F32R, tag="E", bufs=3,
                         name=f"ex{b}_{hp}_{ss}_{c}")
            nc.scalar.activation(ex, st, EXP)
            nc.tensor.matmul(
                ctxA, V[b][c][:, hA * DH:(hA + 1) * DH], ex[:, 0:D],
                start=(c == 0), stop=(c == NT - 1))
            nc.tensor.matmul(
                ctxB, V[b][c][:, hB * DH:(hB + 1) * DH], ex[:, D:S],
                start=(c == 0), stop=(c == NT - 1))

        def ctx_norm(b, hp, ss, ctxA, ctxB, Rt):
            hA, hB = 2 * hp, 2 * hp + 1
            lo, hi = slice(0, DH), slice(DH, P)
            cT = ctxTr[b][hp]
            nc.vector.tensor_mul(
                cT[lo, ss * D:(ss + 1) * D], ctxA,
                Rt[hA][:, ss * D:(ss + 1) * D])
            nc.vector.tensor_mul(
                cT[hi, ss * D:(ss + 1) * D], ctxB,
                Rt[hB][:, ss * D:(ss + 1) * D])

        def build_R(b, hp, scr):
            """Broadcast recip sums to [64, S] rows via DRAM replicate-read."""
            Rt = {}
            for head in (2 * hp, 2 * hp + 1):
                Rh = sb.tile([DH, S], F32, tag="R", bufs=2,
                             name=f"R{b}_{head}")
                flat = scr[head].rearrange("t p -> (t p)")
                src_ap = bass.AP(
                    tensor=flat.tensor, offset=flat.offset,
                    ap=[[0, DH]] + [list(dim) for dim in flat.ap])
                nc.gpsimd.dma_start(Rh, src_ap)
                Rt[head] = Rh
            return Rt

        def phaseD(b):
            for t in range(NT):
                ps = psB.tile([P, D], F32, tag="ps1", bufs=1)
                for c in range(NC):
                    nc.tensor.matmul(
                        ps, ctxTr[b][c][:, t * P:(t + 1) * P], w_sb["wo"][c],
                        start=(c == 0), stop=(c == NC - 1))
                osb = sb.tile([P, D], F32, tag="osb", bufs=2)
                if b == 0:
                    nc.vector.tensor_copy(osb, ps)
                else:
                    nc.scalar.copy(osb, ps)
                nc.sync.dma_start(out_d[b, t * P:(t + 1) * P, :], osb)

        def attn_unit(b, hp):
            rs = {}
            scr = {}
            Rt = {}
            for head in (2 * hp, 2 * hp + 1):
                rs[head] = sb.tile([P, NT], F32, tag="rs", bufs=4,
                                   name=f"rs{b}_{head}")
                scr[head] = dramp.tile([NT, P], F32, name=f"scr{b}_{head}")
                Rt[head] = sb.tile([DH, S], F32, tag="R", bufs=2,
                                   name=f"R{b}_{head}")
            cT = sb.tile([P, S], F32R, tag="ctxT", bufs=6,
                         name=f"cT{b}_{hp}")
            ctxTr[b][hp] = cT
            ctx0 = [psB.tile([DH, D], F32, tag="ctx", bufs=3,
                             name=f"ctx0{b}_{hp}_{i}") for i in range(2)]
            ctx1 = [psB.tile([DH, D], F32, tag="ctx", bufs=3,
                             name=f"ctx1{b}_{hp}_{i}") for i in range(2)]
            for t in range(NT):
                phaseC_tile(b, hp, t, rs, scr, Rt)
                if t >= 6:
                    phaseA_chunk(b, hp, 0, t - 6, ctx0[0], ctx0[1])
            for c in range(2, NT):
                phaseA_chunk(b, hp, 0, c, ctx0[0], ctx0[1])
            for c in range(NT):
                phaseA_chunk(b, hp, 1, c, ctx1[0], ctx1[1])
            ctx_norm(b, hp, 0, ctx0[0], ctx0[1], Rt)
            ctx_norm(b, hp, 1, ctx1[0], ctx1[1], Rt)

        stage01(0)
        attn_unit(0, 0)
        attn_unit(0, 1)
        attn_unit(0, 2)
        attn_unit(0, 3)
        stage01(1)
        attn_unit(1, 0)
        attn_unit(1, 1)
        phaseD(0)
        for hp in range(2, H // 2):
            attn_unit(1, hp)
        phaseD(1)

    nc.compile()
    return nc


def _make_runner(nc):
    """Cached PJRT runner mirroring bass2jax.run_bass_via_pjrt (multi-core)."""
    import jax
    from concourse import bass2jax, mybir
    from jax.experimental.shard_map import shard_map
    from jax.sharding import Mesh, PartitionSpec

    bass2jax.install_neuronx_cc_hook()
    partition_name = (nc.partition_id_tensor.name
                      if nc.partition_id_tensor else None)
    in_names, out_names, out_avals, zero_shapes = [], [], [], []
    for alloc in nc.m.functions[0].allocations:
        if not isinstance(alloc, mybir.MemoryLocationSet):
            continue
        name = alloc.memorylocations[0].name
        if alloc.kind == "ExternalInput":
            if name != partition_name:
                in_names.append(name)
        elif alloc.kind == "ExternalOutput":
            out_names.append(name)
            shape = tuple(alloc.tensor_shape)
            dtype = mybir.dt.np(alloc.dtype)
            out_avals.append(jax.core.ShapedArray(shape, dtype))
            zero_shapes.append((shape, dtype))
    n_params = len(in_names)
    all_in = list(in_names) + list(out_names) + (
        [partition_name] if partition_name else [])
    donate = tuple(range(n_params, n_params + len(out_names)))

    def _body(*args):
        operands = list(args)
        if partition_name:
            operands.append(bass2jax.partition_id_tensor())
        outs = bass2jax._bass_exec_p.bind(
            *operands, out_avals=tuple(out_avals), in_names=tuple(all_in),
            out_names=tuple(out_names), lowering_input_output_aliases=(),
            sim_require_finite=True, sim_require_nnan=True, nc=nc)
        return tuple(outs)

    devices = jax.devices()[:NCORES]
    mesh = Mesh(np.asarray(devices), ("core",))
    specs_in = (PartitionSpec("core"),) * (n_params + len(out_names))
    specs_out = (PartitionSpec("core"),) * len(out_names)
    sharded = jax.jit(
        shard_map(_body, mesh=mesh, in_specs=specs_in, out_specs=specs_out,
                  check_rep=False),
        donate_argnums=donate, keep_unused=True)

    def run(in_maps):
        concat_in = [
            np.concatenate([np.asarray(m[nm]) for m in in_maps], axis=0)
            for nm in in_names]
        concat_zeros = [
            np.zeros((NCORES * sh[0], *sh[1:]), dt) for sh, dt in zero_shapes]
        outs = sharded(*concat_in, *concat_zeros)
        outs = [np.asarray(o) for o in outs]
        return dict(zip(out_names, outs))

    return run


def get_runner():
    if "run" not in _STATE:
        nc = _build_module()
        _STATE["nc"] = nc
        _STATE["run"] = _make_runner(nc)
    return _STATE["run"]


def _numpy_fallback(v, k, q, mask, Wq, Wk, Wv, Wo):
    """Reference implementation in numpy (used only if mask isn't all-ones)."""
    b = q.shape[0]
    scale = 1.0 / np.sqrt(np.float32(DH))

    def split_heads(x):
        return x.reshape(b, S, H, DH).transpose(0, 2, 1, 3)

    qh = split_heads(q @ Wq)
    kh = split_heads(k @ Wk)
    vh = split_heads(v @ Wv)
    logits = np.einsum("bhqd,bhkd->bhqk", qh, kh) * scale
    logits = logits + (1.0 - mask[:, None, None, :]) * (-1e9)
    logits = logits - logits.max(-1, keepdims=True)
    e = np.exp(logits)
    attn = e / e.sum(-1, keepdims=True)
    ctx = np.einsum("bhqk,bhkd->bhqd", attn, vh)
    ctx = ctx.transpose(0, 2, 1, 3).reshape(b, S, D)
    return (ctx @ Wo).astype(np.float32), attn.astype(np.float32)


def kernel(**inputs):
    q = np.asarray(inputs["q"], np.float32)
    k = np.asarray(inputs["k"], np.float32)
    v = np.asarray(inputs["v"], np.float32)
    mask = np.asarray(inputs["mask"], np.float32)
    Wq = np.asarray(inputs["Wq"], np.float32)
    Wk = np.asarray(inputs["Wk"], np.float32)
    Wv = np.asarray(inputs["Wv"], np.float32)
    Wo = np.asarray(inputs["Wo"], np.float32)

    if not np.all(mask == 1.0):
        return _numpy_fallback(v, k, q, mask, Wq, Wk, Wv, Wo)

    run = get_runner()

    qr = np.ascontiguousarray(
        _round_fp32r(q * np.float32(1.0 / np.sqrt(np.float32(DH))))
        .transpose(0, 2, 1))
    kr = np.ascontiguousarray(_round_fp32r(k).transpose(0, 2, 1))
    vr = np.ascontiguousarray(_round_fp32r(v).transpose(0, 2, 1))
    wqr = _round_fp32r(Wq)
    wkr = _round_fp32r(Wk)
    wvr = _round_fp32r(Wv)
    wor = _round_fp32r(Wo)
    in_maps = []
    for c in range(NCORES):
        sl = slice(c * B, (c + 1) * B)
        in_maps.append({
            "q": qr[sl], "k": kr[sl], "v": vr[sl],
            "wq": wqr, "wk": wkr, "wv": wvr, "wo": wor,
        })
    outs = run(in_maps)
    output = outs["out"].reshape(B_FULL, S, D)
    attn = outs["attn"].reshape(B_FULL, H, S, S)
    return output, attn
